# revision 47
# baseline (speedup 1.0000x reference)
"""Trainium2 Bass kernel for GQA attention (b=2, s=2048, d=2048, 16 q heads,
4 kv heads, head_dim=128, causal, RoPE-style freqs) on 8 NeuronCores.

Sharding: 8 cores = 2 batches x 4 kv-head groups. Each core computes, for its
(batch b, group g): the QKV projection for its 4 q heads + 1 kv head, RoPE,
causal attention, and a partial output projection out_part = attn_out @
wo[:, g*512:(g+1)*512].T (contraction-dim shard). The host sums the 4 group
partials per batch.

Design notes (matmul outputs are capped at one 2KB PSUM bank = 512 fp32
columns; instruction count and pipeline continuity dominate real-HW time):
- x/wqkv/wo stream in as bf16 (halves the DMA head); all matmuls run bf16
  at the same 1 column/cycle as f32r but without f32r's 4x penalty on
  <256-col tiles, so the causal diagonal is trimmed at 128-col granularity.
- Phase 1 keeps the whole x tile resident (64KB/partition in bf16) and
  iterates output-chunk-outer: per chunk, 64 matmuls accumulate into a
  [P, 4, 512] PSUM tile (each matmul inside one bank) and ONE full-width
  DVE copy evicts it. RoPE (partition-block rotation via one partition-swap
  DMA + 3 full-width bf16 DVE ops, head_dim deinterleaved on host) runs
  right after each chunk's eviction, hidden under the next chunk's matmuls.
- Softmax skips max-subtraction (scores are O(10); exp is safe in fp32).
  Row sums: per k-chunk pair, DVE adds the two bf16 exp tiles (2x rate)
  into one 512-col tile; a single ones-matmul per pair accumulates the
  partition reduction in PSUM — half the PE cost of per-chunk sums.
  Normalization = DVE reciprocal + multiply at PSUM eviction.
- Attention runs a depth-2 software pipeline: exp of pair k+2 issues before
  pair k is consumed, giving ACT/DVE slack and hiding group-boundary
  normalization under the next group's score matmuls.
- Projection (bf16 attn_out @ bf16 wo) accumulates 4 seq-chunks into a
  [P, 4, 512] PSUM tile, ONE DVE eviction per seq-tile, stores alternate
  the two HWDGE queues.
"""
import os
import sys

for _p in ("/opt/trn_rl_repo", "/root/.axon_site/_ro/trn_rl_repo"):
    if os.path.isdir(_p) and _p not in sys.path:
        sys.path.insert(0, _p)

import numpy as np
from collections import deque
from contextlib import ExitStack

import concourse.bacc as bacc
import concourse.tile as tile
from concourse import mybir
from concourse.bass_utils import run_bass_kernel_spmd
from concourse.masks import make_identity, make_upper_triangular

P = 128
S = 2048            # sequence length
D = 2048            # model dim
HD = 128            # head dim
HQ = 4              # q heads per core
O = 768             # qkv out dims per core (4 q + 1 k + 1 v heads)
NB = 2              # batches
NG = 4              # kv groups
SCALE = float(HD) ** -0.5

f32 = mybir.dt.float32
f32r = mybir.dt.float32r
bf16 = mybir.dt.bfloat16

_NC_CACHE = {}


def build_nc(loop_reps=None, no_rope=False, no_sums=False,
             no_qkv=False, no_proj=False, no_tri=False,
             no_scores=False, trim_scores=True, pend_depth=4,
             ph1_wide=False, proj_wide=False, proj_bf16=True,
             out_bf16=True, padd_eng="pool", dual=True):
    """Build the per-core program. loop_reps wraps the whole compute body in a
    hardware For_i loop (timing only; results are garbage for reps > 1)."""
    nc = bacc.Bacc(trn_type="TRN2", target_bir_lowering=False, debug=False)
    proj_dt = bf16 if proj_bf16 else f32r
    xt = nc.declare_dram_parameter("xt", [D, S], bf16, isOutput=False).ap()
    wqkvt = nc.declare_dram_parameter("wqkvt", [D, O], bf16, isOutput=False).ap()
    wot = nc.declare_dram_parameter("wot", [HQ * HD, D], proj_dt, isOutput=False).ap()
    cos2 = nc.declare_dram_parameter("cos2", [P, S], bf16, isOutput=False).ap()
    sinpm = nc.declare_dram_parameter("sinpm", [P, S], bf16, isOutput=False).ap()
    out = nc.declare_dram_parameter("out", [S, D], bf16 if out_bf16 else f32,
                                    isOutput=True).ap()

    with tile.TileContext(nc) as tc, ExitStack() as outer:
        const = outer.enter_context(tc.tile_pool(name="const", bufs=1))
        qkvp = outer.enter_context(tc.tile_pool(name="qkvp", bufs=1))

        # constants
        cos_t = const.tile([P, S], bf16)
        sin_t = const.tile([P, S], bf16)
        nc.gpsimd.dma_start(out=cos_t, in_=cos2)
        nc.gpsimd.dma_start(out=sin_t, in_=sinpm)
        ident = const.tile([P, P], f32)
        make_identity(nc, ident)
        tri = const.tile([P, P], bf16)
        make_upper_triangular(nc, tri, val=1.0, diag=True)
        ones = const.tile([P, P], bf16)
        nc.vector.memset(ones, 1.0)

        # persistent activations
        qkvT = qkvp.tile([P, 5, S], bf16)       # q heads 0-3 + k, [d|128, c, s]
        vT = qkvp.tile([P, S], f32r)            # v, [d|128, s]
        wo_t = qkvp.tile([P, HQ, D], proj_dt)
        nc.gpsimd.dma_start(out=wo_t, in_=wot.rearrange("(c p) o -> p c o", p=P))
        attn_outT = qkvp.tile([P, HQ, S], proj_dt)   # [d|128, head, s]
        kT = qkvT[:, 4, :]

        loop_cm = tc.For_i(
            0, loop_reps, 1,
            hint_engines=(mybir.EngineType.PE, mybir.EngineType.Activation,
                          mybir.EngineType.DVE, mybir.EngineType.SP,
                          mybir.EngineType.Pool)) if loop_reps is not None else None
        if loop_cm is not None:
            loop_cm.__enter__()

        OC_ORDER = (4, 0, 1, 2, 3, 5)

        # ---- Phase 1: QKV projection + RoPE ----
        with ExitStack() as ph1:
            wq_pool = ph1.enter_context(tc.tile_pool(name="wq", bufs=1))
            xt_pool = ph1.enter_context(
                tc.tile_pool(name="xtp", bufs=1 if ph1_wide else 2))
            swp_pool = ph1.enter_context(
                tc.tile_pool(name="swp", bufs=1 if ph1_wide else 2))
            ps1 = ph1.enter_context(tc.tile_pool(name="ps1", bufs=2, space="PSUM"))
            wq_t = wq_pool.tile([P, 16, O], bf16)
            wq_src = wqkvt.rearrange("(c p) o -> p c o", p=P)
            xt_src = xt.rearrange("(c p) s -> p c s", p=P)

            def wq_load(eng, oc):
                eng.dma_start(out=wq_t[:, :, oc * P:(oc + 1) * P],
                              in_=wq_src[:, :, oc * P:(oc + 1) * P])

            def rope_chunk(c, half=None):
                src = qkvT[:, c, :] if c < 4 else kT
                a, b = (0, S) if half is None else (half * 1024, half * 1024 + 1024)
                src = src[:, a:b]
                swp = swp_pool.tile([P, S if half is None else 1024], bf16)
                nc.gpsimd.dma_start(out=swp[0:64, :], in_=src[64:128, :])
                nc.gpsimd.dma_start(out=swp[64:128, :], in_=src[0:64, :])
                nc.vector.tensor_mul(swp, swp, sin_t[:, a:b])
                nc.vector.tensor_mul(src, src, cos_t[:, a:b])
                nc.vector.tensor_add(src, src, swp)

            if ph1_wide:
                # x resident, chunk-outer, one wide eviction per chunk
                xt_t = xt_pool.tile([P, 16, S], bf16)
                wq_load(nc.sync, OC_ORDER[0])
                wq_load(nc.scalar, OC_ORDER[1])
                for dc in range(16):
                    eng = nc.sync if dc % 2 == 0 else nc.scalar
                    eng.dma_start(out=xt_t[:, dc, :], in_=xt_src[:, dc, :])
                    if dc == 7:
                        wq_load(nc.sync, OC_ORDER[2])
                        wq_load(nc.scalar, OC_ORDER[3])
                wq_load(nc.sync, OC_ORDER[4])
                wq_load(nc.scalar, OC_ORDER[5])
                for oc in OC_ORDER:
                    if no_qkv:
                        break
                    pt = ps1.tile([P, 4, 512], f32)
                    for sb in range(4):
                        for dc in range(16):
                            nc.tensor.matmul(
                                pt[:, sb, :], wq_t[:, dc, oc * P:(oc + 1) * P],
                                xt_t[:, dc, sb * 512:(sb + 1) * 512],
                                start=(dc == 0), stop=(dc == 15))
                    if oc < 4:
                        nc.vector.tensor_copy(qkvT[:, oc, :], pt)
                    elif oc == 4:
                        nc.vector.tensor_copy(kT, pt)
                    else:
                        nc.vector.tensor_copy(vT, pt)
                    if oc != 5 and not no_rope:
                        rope_chunk(oc)
            else:
                # seq-block outer (v1 style): per-sb x tiles, narrow evictions
                for oc in range(6):
                    wq_load(nc.scalar, oc)
                for sb in range(4):
                    xt_t = xt_pool.tile([P, 16, 512], bf16)
                    src = xt_src[:, :, sb * 512:(sb + 1) * 512]
                    if sb == 0:
                        for q4 in range(4):
                            nc.sync.dma_start(
                                out=xt_t[:, q4 * 4:(q4 + 1) * 4, :],
                                in_=src[:, q4 * 4:(q4 + 1) * 4, :])
                    else:
                        eng = nc.sync if sb % 2 == 0 else nc.scalar
                        eng.dma_start(out=xt_t, in_=src)
                    for oc in range(0 if no_qkv else 6):
                        pt = ps1.tile([P, 4, 512], f32, name="pt")
                        ptb = pt[:, oc % 4, :]
                        for dc in range(16):
                            nc.tensor.matmul(
                                ptb, wq_t[:, dc, oc * P:(oc + 1) * P],
                                xt_t[:, dc, :],
                                start=(dc == 0), stop=(dc == 15))
                        sl = slice(sb * 512, (sb + 1) * 512)
                        if oc < 4:
                            nc.vector.tensor_copy(qkvT[:, oc, sl], ptb)
                        elif oc == 4:
                            nc.vector.tensor_copy(kT[:, sl], ptb)
                        else:
                            nc.vector.tensor_copy(vT[:, sl], ptb)
                    if sb in (1, 3) and not no_rope:
                        for c in (4, 0, 1, 2, 3):
                            rope_chunk(c, half=sb // 2)

        # ---- Phase 2: V build + attention ----
        with ExitStack() as ph2:
            vpool = ph2.enter_context(tc.tile_pool(name="vpool", bufs=1))
            attn_pool = ph2.enter_context(tc.tile_pool(name="attn", bufs=2 + pend_depth))
            padd_pool = ph2.enter_context(
                tc.tile_pool(name="padd", bufs=pend_depth + 2))
            recb_pool = ph2.enter_context(tc.tile_pool(name="recb", bufs=2))
            V = vpool.tile([P, 16, HD], bf16)              # [s|128, s-chunk, d]

            with ExitStack() as psx:
                ps_sc = psx.enter_context(
                    tc.tile_pool(name="ps_sc", bufs=2, space="PSUM"))
                ps_acc = psx.enter_context(
                    tc.tile_pool(name="ps_acc", bufs=2, space="PSUM"))
                ps_sum = psx.enter_context(
                    tc.tile_pool(name="ps_sum", bufs=2, space="PSUM"))

                # V build: PE-transpose of vT ([d, s] -> [s, d])
                for t in range(16):
                    tp_full = ps_sc.tile([P, 1024], f32, tag="sc", name="tp")
                    tp = tp_full[:, :P]
                    nc.tensor.transpose(
                        tp, vT[:, t * P:(t + 1) * P].bitcast(f32), ident)
                    nc.vector.tensor_copy(V[:, t, :], tp)

                def emit_sum(s_sum, padd, jdA, is_start, is_stop):
                    nc.tensor.matmul(
                        s_sum[:, jdA * P:512], ones, padd[:, jdA * P:512],
                        start=is_start, stop=is_stop)

                def flush_group(st):
                    g, h, o_ps, s_sum, sums_q = st
                    while sums_q:
                        emit_sum(s_sum, *sums_q.popleft())
                    dst = attn_outT[:, h, g * 512:(g + 1) * 512]
                    if no_sums:
                        nc.vector.tensor_copy(dst, o_ps)
                    else:
                        recb = recb_pool.tile([P, 512], f32)
                        nc.vector.reciprocal(recb, s_sum)
                        nc.vector.tensor_mul(dst, o_ps, recb)

                fin = {"cur": None}

                def consume(at2, kcp, g, h, o_ps, s_sum, sums_q, nkc):
                    if fin["cur"] is not None and fin["cur"][:2] != (g, h):
                        flush_group(fin["cur"])
                        fin["cur"] = None
                    kcA, kcB = 2 * kcp, 2 * kcp + 1
                    jdA = max(0, kcA - 4 * g)
                    jdB = max(0, kcB - 4 * g)
                    # PE: attn@V for both blocks + the pair's sums matmul
                    # (mask/pair-add ran at issue time, two pairs ago)
                    nc.tensor.matmul(
                        o_ps[:, jdA * P:512], V[:, kcA, :],
                        at2[:, jdA * P:512],
                        start=(kcA == 0), stop=False)
                    nc.tensor.matmul(
                        o_ps[:, jdB * P:512], V[:, kcB, :],
                        at2[:, 512 + jdB * P:1024],
                        start=False, stop=(kcB == nkc - 1))
                    if not no_sums:
                        emit_sum(s_sum, *sums_q.popleft())
                    if kcB == nkc - 1:   # group finished: finalize later
                        fin["cur"] = (g, h, o_ps, s_sum, sums_q)

                def issue_pair(g, h, kcp, o_ps, s_sum, sums_q, pend):
                    nkc = 4 * (g + 1)
                    qs = g * 512
                    kcA, kcB = 2 * kcp, 2 * kcp + 1
                    # trim the first score matmul of a diagonal pair; keep the
                    # second full so one contiguous exp covers the pair (the
                    # gap columns are computed-but-unread, saving an ACT call)
                    jdA = max(0, kcA - 4 * g) if trim_scores else 0
                    s2 = ps_sc.tile([P, 1024], f32, tag="sc", name="s2")
                    nc.tensor.matmul(
                        s2[:, jdA * P:512],
                        kT[:, kcA * P:(kcA + 1) * P],
                        qkvT[:, h, qs + jdA * P:qs + 512],
                        start=True, stop=True)
                    nc.tensor.matmul(
                        s2[:, 512:1024],
                        kT[:, kcB * P:(kcB + 1) * P],
                        qkvT[:, h, qs:qs + 512],
                        start=True, stop=True)
                    at2 = attn_pool.tile([P, 1024], bf16)
                    nc.scalar.activation(
                        out=at2[:, jdA * P:1024], in_=s2[:, jdA * P:1024],
                        func=mybir.ActivationFunctionType.Exp, scale=SCALE)
                    diag = kcA >= 4 * g
                    jtA = max(0, kcA - 4 * g)
                    jtB = max(0, kcB - 4 * g)
                    if diag and not no_tri:
                        nc.vector.tensor_mul(
                            at2[:, jtA * P:(jtA + 1) * P],
                            at2[:, jtA * P:(jtA + 1) * P], tri)
                        nc.vector.tensor_mul(
                            at2[:, 512 + jtB * P:512 + (jtB + 1) * P],
                            at2[:, 512 + jtB * P:512 + (jtB + 1) * P], tri)
                    if not no_sums:
                        padd = padd_pool.tile([P, 512], bf16)
                        peng = (nc.gpsimd if padd_eng == "pool"
                                else nc.vector if padd_eng == "dve"
                                else (nc.gpsimd if kcp % 2 == 0 else nc.vector))
                        if diag:
                            peng.tensor_copy(
                                padd[:, jtA * P:jtB * P],
                                at2[:, jtA * P:jtB * P])
                            peng.tensor_add(
                                padd[:, jtB * P:512], at2[:, jtB * P:512],
                                at2[:, 512 + jtB * P:1024])
                        else:
                            peng.tensor_add(
                                padd, at2[:, 0:512], at2[:, 512:1024])
                        sums_q.append((padd, jtA, kcA == 0, kcB == nkc - 1))
                    pend.append((at2, kcp, g, h, o_ps, s_sum, sums_q, nkc))
                    if len(pend) > pend_depth:
                        consume(*pend.popleft())

                if not no_scores:
                    pend = deque()
                    for g in range(NG):
                        nkc = 4 * (g + 1)
                        # two same-length head groups interleaved at pair
                        # granularity: each exp gets a full extra pair of
                        # slack before the PE needs its PSUM slot back
                        GP = 2 if dual else 1
                        for hp in range(HQ // GP):
                            ctxs = []
                            for hh in range(GP * hp, GP * hp + GP):
                                ctxs.append((hh,
                                             ps_acc.tile([P, 512], f32,
                                                         name="o_ps"),
                                             ps_sum.tile([P, 512], f32,
                                                         name="s_sum"),
                                             deque()))
                            for kcp in range(nkc // 2):
                                for hh, o_ps, s_sum, sums_q in ctxs:
                                    issue_pair(g, hh, kcp, o_ps, s_sum,
                                               sums_q, pend)
                    while pend:
                        consume(*pend.popleft())
                    if fin["cur"] is not None:
                        flush_group(fin["cur"])
                        fin["cur"] = None

        # ---- Phase 3: output projection ----
        if not no_proj:
            with ExitStack() as ph3:
                oev_pool = ph3.enter_context(tc.tile_pool(name="oev", bufs=3))
                ps_pp = ph3.enter_context(
                    tc.tile_pool(name="ps_pp", bufs=2 if proj_wide else 4,
                                 space="PSUM"))
                for st in range(16):
                    ot = oev_pool.tile([P, D], bf16 if out_bf16 else f32)
                    if proj_wide:
                        pp = ps_pp.tile([P, 4, 512], f32)
                        for oc in range(4):
                            for h2 in range(HQ):
                                nc.tensor.matmul(
                                    pp[:, oc, :],
                                    attn_outT[:, h2, st * P:(st + 1) * P],
                                    wo_t[:, h2, oc * 512:(oc + 1) * 512],
                                    start=(h2 == 0), stop=(h2 == 3))
                        nc.vector.tensor_copy(ot, pp)
                    else:
                        for oc in range(4):
                            pp = ps_pp.tile([P, 512], f32, name="pp")
                            for h2 in range(HQ):
                                nc.tensor.matmul(
                                    pp, attn_outT[:, h2, st * P:(st + 1) * P],
                                    wo_t[:, h2, oc * 512:(oc + 1) * 512],
                                    start=(h2 == 0), stop=(h2 == 3))
                            nc.vector.tensor_copy(
                                ot[:, oc * 512:(oc + 1) * 512], pp)
                    eng = nc.scalar if st % 2 == 0 else nc.sync
                    eng.dma_start(out=out[st * P:(st + 1) * P, :], in_=ot)

        if loop_cm is not None:
            loop_cm.__exit__(None, None, None)

    nc.compile()
    return nc


def _to_bf16(a):
    import ml_dtypes
    return np.ascontiguousarray(a.astype(ml_dtypes.bfloat16))


def _prep_inputs(x, freqs_cis, wqkv, wo, proj_bf16=True):
    """Host-side sharding/layout prep. Returns in_maps for cores b*4+g."""
    x = np.ascontiguousarray(np.asarray(x, dtype=np.float32))
    freqs_cis = np.asarray(freqs_cis, dtype=np.float32)
    wqkv = np.asarray(wqkv, dtype=np.float32)
    wo = np.asarray(wo, dtype=np.float32)

    perm = np.concatenate([np.arange(0, HD, 2), np.arange(1, HD, 2)])
    wq = wqkv[:D].reshape(16, HD, D)[:, perm, :]
    wk = wqkv[D:D + 512].reshape(4, HD, D)[:, perm, :]
    wv = wqkv[D + 512:].reshape(4, HD, D)

    cosT = freqs_cis[:, :, 0].T            # [64, S]
    sinT = freqs_cis[:, :, 1].T
    cos2 = _to_bf16(np.concatenate([cosT, cosT], axis=0))
    sinpm = _to_bf16(np.concatenate([-sinT, sinT], axis=0))

    xts = [_to_bf16(x[b].T) for b in range(NB)]
    in_maps = []
    for b in range(NB):
        for g in range(NG):
            wshard = np.concatenate(
                [wq[g * 4 + h] for h in range(4)] + [wk[g], wv[g]], axis=0)
            wqkvt = _to_bf16(wshard.T)
            wot = (_to_bf16(wo[:, g * 512:(g + 1) * 512].T) if proj_bf16
                   else np.ascontiguousarray(wo[:, g * 512:(g + 1) * 512].T))
            in_maps.append({"xt": xts[b], "wqkvt": wqkvt, "wot": wot,
                            "cos2": cos2, "sinpm": sinpm})
    return in_maps


def kernel(x, freqs_cis, wqkv, wo):
    if "main" not in _NC_CACHE:
        _NC_CACHE["main"] = build_nc()
    nc = _NC_CACHE["main"]
    in_maps = _prep_inputs(x, freqs_cis, wqkv, wo)
    res = run_bass_kernel_spmd(nc, in_maps, list(range(NB * NG)))
    out = np.zeros((NB, S, D), dtype=np.float32)
    for b in range(NB):
        for g in range(NG):
            out[b] += res.results[b * NG + g]["out"].astype(np.float32)
    return out


# revision 52
# speedup vs baseline: 1.0281x; 1.0281x over previous
"""Trainium2 Bass kernel for GQA attention (b=2, s=2048, d=2048, 16 q heads,
4 kv heads, head_dim=128, causal, RoPE-style freqs) on 8 NeuronCores.

Sharding: 8 cores = 2 batches x 4 kv-head groups. Each core computes, for its
(batch b, group g): the QKV projection for its 4 q heads + 1 kv head, RoPE,
causal attention, and a partial output projection out_part = attn_out @
wo[:, g*512:(g+1)*512].T (contraction-dim shard). The host sums the 4 group
partials per batch.

Design notes (matmul outputs are capped at one 2KB PSUM bank = 512 fp32
columns; instruction count and pipeline continuity dominate real-HW time):
- x/wqkv/wo stream in as bf16 (halves the DMA head); all matmuls run bf16
  at the same 1 column/cycle as f32r but without f32r's 4x penalty on
  <256-col tiles, so the causal diagonal is trimmed at 128-col granularity.
- Phase 1 keeps the whole x tile resident (64KB/partition in bf16) and
  iterates output-chunk-outer: per chunk, 64 matmuls accumulate into a
  [P, 4, 512] PSUM tile (each matmul inside one bank) and ONE full-width
  DVE copy evicts it. RoPE (partition-block rotation via one partition-swap
  DMA + 3 full-width bf16 DVE ops, head_dim deinterleaved on host) runs
  right after each chunk's eviction, hidden under the next chunk's matmuls.
- Softmax skips max-subtraction (scores are O(10); exp is safe in fp32).
  Row sums: per k-chunk pair, DVE adds the two bf16 exp tiles (2x rate)
  into one 512-col tile; a single ones-matmul per pair accumulates the
  partition reduction in PSUM — half the PE cost of per-chunk sums.
  Normalization = DVE reciprocal + multiply at PSUM eviction.
- Attention runs a depth-2 software pipeline: exp of pair k+2 issues before
  pair k is consumed, giving ACT/DVE slack and hiding group-boundary
  normalization under the next group's score matmuls.
- Projection (bf16 attn_out @ bf16 wo) accumulates 4 seq-chunks into a
  [P, 4, 512] PSUM tile, ONE DVE eviction per seq-tile, stores alternate
  the two HWDGE queues.
"""
import os
import sys

for _p in ("/opt/trn_rl_repo", "/root/.axon_site/_ro/trn_rl_repo"):
    if os.path.isdir(_p) and _p not in sys.path:
        sys.path.insert(0, _p)

import numpy as np
from collections import deque
from contextlib import ExitStack

import concourse.bacc as bacc
import concourse.tile as tile
from concourse import mybir
from concourse.bass_utils import run_bass_kernel_spmd
from concourse.masks import make_identity, make_upper_triangular

P = 128
S = 2048            # sequence length
D = 2048            # model dim
HD = 128            # head dim
HQ = 4              # q heads per core
O = 768             # qkv out dims per core (4 q + 1 k + 1 v heads)
NB = 2              # batches
NG = 4              # kv groups
SCALE = float(HD) ** -0.5

f32 = mybir.dt.float32
f32r = mybir.dt.float32r
bf16 = mybir.dt.bfloat16

_NC_CACHE = {}


def build_nc(loop_reps=None, no_rope=False, no_sums=False,
             no_qkv=False, no_proj=False, no_tri=False,
             no_scores=False, trim_scores=True, pend_depth=6,
             ph1_wide=False, proj_wide=False, proj_bf16=True,
             out_bf16=True, padd_eng="pool", dual=True):
    """Build the per-core program. loop_reps wraps the whole compute body in a
    hardware For_i loop (timing only; results are garbage for reps > 1)."""
    nc = bacc.Bacc(trn_type="TRN2", target_bir_lowering=False, debug=False)
    proj_dt = bf16 if proj_bf16 else f32r
    xt = nc.declare_dram_parameter("xt", [D, S], bf16, isOutput=False).ap()
    wqkvt = nc.declare_dram_parameter("wqkvt", [D, O], bf16, isOutput=False).ap()
    wot = nc.declare_dram_parameter("wot", [HQ * HD, D], proj_dt, isOutput=False).ap()
    cos2 = nc.declare_dram_parameter("cos2", [P, S], bf16, isOutput=False).ap()
    sinpm = nc.declare_dram_parameter("sinpm", [P, S], bf16, isOutput=False).ap()
    out = nc.declare_dram_parameter("out", [S, D], bf16 if out_bf16 else f32,
                                    isOutput=True).ap()

    with tile.TileContext(nc) as tc, ExitStack() as outer:
        const = outer.enter_context(tc.tile_pool(name="const", bufs=1))
        qkvp = outer.enter_context(tc.tile_pool(name="qkvp", bufs=1))

        # constants
        cos_t = const.tile([P, S], bf16)
        sin_t = const.tile([P, S], bf16)
        nc.gpsimd.dma_start(out=cos_t, in_=cos2)
        nc.gpsimd.dma_start(out=sin_t, in_=sinpm)
        ident = const.tile([P, P], f32)
        make_identity(nc, ident)
        tri = const.tile([P, P], bf16)
        make_upper_triangular(nc, tri, val=1.0, diag=True)
        ones = const.tile([P, P], bf16)
        nc.vector.memset(ones, 1.0)

        # persistent activations
        qkvT = qkvp.tile([P, 5, S], bf16)       # q heads 0-3 + k, [d|128, c, s]
        vT = qkvp.tile([P, S], f32r)            # v, [d|128, s]
        wo_t = qkvp.tile([P, HQ, D], proj_dt)
        nc.gpsimd.dma_start(out=wo_t, in_=wot.rearrange("(c p) o -> p c o", p=P))
        attn_outT = qkvp.tile([P, HQ, S], proj_dt)   # [d|128, head, s]
        V = qkvp.tile([P, 16, HD], bf16)             # [s|128, s-chunk, d]
        kT = qkvT[:, 4, :]

        loop_cm = tc.For_i(
            0, loop_reps, 1,
            hint_engines=(mybir.EngineType.PE, mybir.EngineType.Activation,
                          mybir.EngineType.DVE, mybir.EngineType.SP,
                          mybir.EngineType.Pool)) if loop_reps is not None else None
        if loop_cm is not None:
            loop_cm.__enter__()

        OC_ORDER = (4, 0, 1, 2, 3, 5)

        # ---- Phase 1: QKV projection + RoPE ----
        with ExitStack() as ph1:
            wq_pool = ph1.enter_context(tc.tile_pool(name="wq", bufs=1))
            xt_pool = ph1.enter_context(
                tc.tile_pool(name="xtp", bufs=1 if ph1_wide else 2))
            swp_pool = ph1.enter_context(
                tc.tile_pool(name="swp", bufs=1 if ph1_wide else 2))
            ps1 = ph1.enter_context(tc.tile_pool(name="ps1", bufs=2, space="PSUM"))
            wq_t = wq_pool.tile([P, 16, O], bf16)
            wq_src = wqkvt.rearrange("(c p) o -> p c o", p=P)
            xt_src = xt.rearrange("(c p) s -> p c s", p=P)

            def wq_load(eng, oc):
                eng.dma_start(out=wq_t[:, :, oc * P:(oc + 1) * P],
                              in_=wq_src[:, :, oc * P:(oc + 1) * P])

            def rope_chunk(c, half=None):
                src = qkvT[:, c, :] if c < 4 else kT
                a, b = (0, S) if half is None else (half * 1024, half * 1024 + 1024)
                src = src[:, a:b]
                swp = swp_pool.tile([P, S if half is None else 1024], bf16)
                nc.gpsimd.dma_start(out=swp[0:64, :], in_=src[64:128, :])
                nc.gpsimd.dma_start(out=swp[64:128, :], in_=src[0:64, :])
                nc.vector.tensor_mul(swp, swp, sin_t[:, a:b])
                nc.vector.tensor_mul(src, src, cos_t[:, a:b])
                nc.vector.tensor_add(src, src, swp)

            if ph1_wide:
                # x resident, chunk-outer, one wide eviction per chunk
                xt_t = xt_pool.tile([P, 16, S], bf16)
                wq_load(nc.sync, OC_ORDER[0])
                wq_load(nc.scalar, OC_ORDER[1])
                for dc in range(16):
                    eng = nc.sync if dc % 2 == 0 else nc.scalar
                    eng.dma_start(out=xt_t[:, dc, :], in_=xt_src[:, dc, :])
                    if dc == 7:
                        wq_load(nc.sync, OC_ORDER[2])
                        wq_load(nc.scalar, OC_ORDER[3])
                wq_load(nc.sync, OC_ORDER[4])
                wq_load(nc.scalar, OC_ORDER[5])
                for oc in OC_ORDER:
                    if no_qkv:
                        break
                    pt = ps1.tile([P, 4, 512], f32)
                    for sb in range(4):
                        for dc in range(16):
                            nc.tensor.matmul(
                                pt[:, sb, :], wq_t[:, dc, oc * P:(oc + 1) * P],
                                xt_t[:, dc, sb * 512:(sb + 1) * 512],
                                start=(dc == 0), stop=(dc == 15))
                    if oc < 4:
                        nc.vector.tensor_copy(qkvT[:, oc, :], pt)
                    elif oc == 4:
                        nc.vector.tensor_copy(kT, pt)
                    else:
                        nc.vector.tensor_copy(vT, pt)
                    if oc != 5 and not no_rope:
                        rope_chunk(oc)
            else:
                # seq-block outer (v1 style): per-sb x tiles, narrow evictions
                for oc in range(6):
                    wq_load(nc.scalar, oc)
                for sb in range(4):
                    xt_t = xt_pool.tile([P, 16, 512], bf16)
                    src = xt_src[:, :, sb * 512:(sb + 1) * 512]
                    if sb == 0:
                        for q4 in range(4):
                            nc.sync.dma_start(
                                out=xt_t[:, q4 * 4:(q4 + 1) * 4, :],
                                in_=src[:, q4 * 4:(q4 + 1) * 4, :])
                    else:
                        eng = nc.sync if sb % 2 == 0 else nc.scalar
                        eng.dma_start(out=xt_t, in_=src)
                    for oc in range(0 if no_qkv else 6):
                        pt = ps1.tile([P, 4, 512], f32, name="pt")
                        ptb = pt[:, oc % 4, :]
                        for dc in range(16):
                            nc.tensor.matmul(
                                ptb, wq_t[:, dc, oc * P:(oc + 1) * P],
                                xt_t[:, dc, :],
                                start=(dc == 0), stop=(dc == 15))
                        sl = slice(sb * 512, (sb + 1) * 512)
                        if oc < 4:
                            nc.vector.tensor_copy(qkvT[:, oc, sl], ptb)
                        elif oc == 4:
                            nc.vector.tensor_copy(kT[:, sl], ptb)
                        else:
                            nc.vector.tensor_copy(vT[:, sl], ptb)
                    if sb in (1, 3) and not no_rope:
                        for c in (4, 0, 1, 2, 3):
                            rope_chunk(c, half=sb // 2)

            # V build in the phase-1 tail: PE-transpose of vT ([d,s]->[s,d]),
            # 4 transposes per PSUM tile + one 2D-AP eviction each
            if not no_qkv:
                for t4 in range(4):
                    tp = ps1.tile([P, 4, 512], f32, name="pt")
                    for q in range(4):
                        t = t4 * 4 + q
                        nc.tensor.transpose(
                            tp[:, q, :P], vT[:, t * P:(t + 1) * P].bitcast(f32),
                            ident)
                    nc.vector.tensor_copy(
                        V[:, t4 * 4:(t4 + 1) * 4, :], tp[:, :, :P])

        # ---- Phase 2: V build + attention ----
        with ExitStack() as ph2:
            attn_pool = ph2.enter_context(tc.tile_pool(name="attn", bufs=2 + pend_depth))
            padd_pool = ph2.enter_context(
                tc.tile_pool(name="padd", bufs=pend_depth + 2))
            recb_pool = ph2.enter_context(tc.tile_pool(name="recb", bufs=2))

            with ExitStack() as psx:
                ps_sc = psx.enter_context(
                    tc.tile_pool(name="ps_sc", bufs=2, space="PSUM"))
                ps_acc = psx.enter_context(
                    tc.tile_pool(name="ps_acc", bufs=2, space="PSUM"))
                ps_sum = psx.enter_context(
                    tc.tile_pool(name="ps_sum", bufs=2, space="PSUM"))

                def emit_sum(s_sum, padd, jdA, is_start, is_stop):
                    nc.tensor.matmul(
                        s_sum[:, jdA * P:512], ones, padd[:, jdA * P:512],
                        start=is_start, stop=is_stop)

                def flush_group(st):
                    g, h, o_ps, s_sum, sums_q = st
                    while sums_q:
                        emit_sum(s_sum, *sums_q.popleft())
                    dst = attn_outT[:, h, g * 512:(g + 1) * 512]
                    if no_sums:
                        nc.vector.tensor_copy(dst, o_ps)
                    else:
                        recb = recb_pool.tile([P, 512], f32)
                        nc.vector.reciprocal(recb, s_sum)
                        nc.vector.tensor_mul(dst, o_ps, recb)

                fin = {"cur": None}

                def consume(at2, kcp, g, h, o_ps, s_sum, sums_q, nkc):
                    if fin["cur"] is not None and fin["cur"][:2] != (g, h):
                        flush_group(fin["cur"])
                        fin["cur"] = None
                    kcA, kcB = 2 * kcp, 2 * kcp + 1
                    jdA = max(0, kcA - 4 * g)
                    jdB = max(0, kcB - 4 * g)
                    # PE: attn@V for both blocks + the pair's sums matmul
                    # (mask/pair-add ran at issue time, two pairs ago)
                    nc.tensor.matmul(
                        o_ps[:, jdA * P:512], V[:, kcA, :],
                        at2[:, jdA * P:512],
                        start=(kcA == 0), stop=False)
                    nc.tensor.matmul(
                        o_ps[:, jdB * P:512], V[:, kcB, :],
                        at2[:, 512 + jdB * P:1024],
                        start=False, stop=(kcB == nkc - 1))
                    if not no_sums:
                        emit_sum(s_sum, *sums_q.popleft())
                    if kcB == nkc - 1:   # group finished: finalize later
                        fin["cur"] = (g, h, o_ps, s_sum, sums_q)

                def issue_pair(g, h, kcp, o_ps, s_sum, sums_q, pend):
                    nkc = 4 * (g + 1)
                    qs = g * 512
                    kcA, kcB = 2 * kcp, 2 * kcp + 1
                    # trim the first score matmul of a diagonal pair; keep the
                    # second full so one contiguous exp covers the pair (the
                    # gap columns are computed-but-unread, saving an ACT call)
                    jdA = max(0, kcA - 4 * g) if trim_scores else 0
                    s2 = ps_sc.tile([P, 1024], f32, tag="sc", name="s2")
                    nc.tensor.matmul(
                        s2[:, jdA * P:512],
                        kT[:, kcA * P:(kcA + 1) * P],
                        qkvT[:, h, qs + jdA * P:qs + 512],
                        start=True, stop=True)
                    nc.tensor.matmul(
                        s2[:, 512:1024],
                        kT[:, kcB * P:(kcB + 1) * P],
                        qkvT[:, h, qs:qs + 512],
                        start=True, stop=True)
                    at2 = attn_pool.tile([P, 1024], bf16)
                    nc.scalar.activation(
                        out=at2[:, jdA * P:1024], in_=s2[:, jdA * P:1024],
                        func=mybir.ActivationFunctionType.Exp, scale=SCALE)
                    diag = kcA >= 4 * g
                    jtA = max(0, kcA - 4 * g)
                    jtB = max(0, kcB - 4 * g)
                    if diag and not no_tri:
                        nc.vector.tensor_mul(
                            at2[:, jtA * P:(jtA + 1) * P],
                            at2[:, jtA * P:(jtA + 1) * P], tri)
                        nc.vector.tensor_mul(
                            at2[:, 512 + jtB * P:512 + (jtB + 1) * P],
                            at2[:, 512 + jtB * P:512 + (jtB + 1) * P], tri)
                    if not no_sums:
                        padd = padd_pool.tile([P, 512], bf16)
                        peng = (nc.gpsimd if padd_eng == "pool"
                                else nc.vector if padd_eng == "dve"
                                else (nc.gpsimd if kcp % 2 == 0 else nc.vector))
                        if diag:
                            peng.tensor_copy(
                                padd[:, jtA * P:jtB * P],
                                at2[:, jtA * P:jtB * P])
                            peng.tensor_add(
                                padd[:, jtB * P:512], at2[:, jtB * P:512],
                                at2[:, 512 + jtB * P:1024])
                        else:
                            peng.tensor_add(
                                padd, at2[:, 0:512], at2[:, 512:1024])
                        sums_q.append((padd, jtA, kcA == 0, kcB == nkc - 1))
                    pend.append((at2, kcp, g, h, o_ps, s_sum, sums_q, nkc))
                    if len(pend) > pend_depth:
                        consume(*pend.popleft())

                if not no_scores:
                    pend = deque()
                    for g in range(NG):
                        nkc = 4 * (g + 1)
                        # two same-length head groups interleaved at pair
                        # granularity: each exp gets a full extra pair of
                        # slack before the PE needs its PSUM slot back
                        GP = 2 if dual else 1
                        for hp in range(HQ // GP):
                            ctxs = []
                            for hh in range(GP * hp, GP * hp + GP):
                                ctxs.append((hh,
                                             ps_acc.tile([P, 512], f32,
                                                         name="o_ps"),
                                             ps_sum.tile([P, 512], f32,
                                                         name="s_sum"),
                                             deque()))
                            for kcp in range(nkc // 2):
                                for hh, o_ps, s_sum, sums_q in ctxs:
                                    issue_pair(g, hh, kcp, o_ps, s_sum,
                                               sums_q, pend)
                    while pend:
                        consume(*pend.popleft())
                    if fin["cur"] is not None:
                        flush_group(fin["cur"])
                        fin["cur"] = None

        # ---- Phase 3: output projection ----
        if not no_proj:
            with ExitStack() as ph3:
                oev_pool = ph3.enter_context(tc.tile_pool(name="oev", bufs=3))
                ps_pp = ph3.enter_context(
                    tc.tile_pool(name="ps_pp", bufs=2 if proj_wide else 4,
                                 space="PSUM"))
                for st in range(16):
                    ot = oev_pool.tile([P, D], bf16 if out_bf16 else f32)
                    if proj_wide:
                        pp = ps_pp.tile([P, 4, 512], f32)
                        for oc in range(4):
                            for h2 in range(HQ):
                                nc.tensor.matmul(
                                    pp[:, oc, :],
                                    attn_outT[:, h2, st * P:(st + 1) * P],
                                    wo_t[:, h2, oc * 512:(oc + 1) * 512],
                                    start=(h2 == 0), stop=(h2 == 3))
                        nc.vector.tensor_copy(ot, pp)
                    else:
                        for oc in range(4):
                            pp = ps_pp.tile([P, 512], f32, name="pp")
                            for h2 in range(HQ):
                                nc.tensor.matmul(
                                    pp, attn_outT[:, h2, st * P:(st + 1) * P],
                                    wo_t[:, h2, oc * 512:(oc + 1) * 512],
                                    start=(h2 == 0), stop=(h2 == 3))
                            nc.vector.tensor_copy(
                                ot[:, oc * 512:(oc + 1) * 512], pp)
                    eng = nc.scalar if st % 2 == 0 else nc.sync
                    eng.dma_start(out=out[st * P:(st + 1) * P, :], in_=ot)

        if loop_cm is not None:
            loop_cm.__exit__(None, None, None)

    nc.compile()
    return nc


def _to_bf16(a):
    import ml_dtypes
    return np.ascontiguousarray(a.astype(ml_dtypes.bfloat16))


def _prep_inputs(x, freqs_cis, wqkv, wo, proj_bf16=True):
    """Host-side sharding/layout prep. Returns in_maps for cores b*4+g."""
    x = np.ascontiguousarray(np.asarray(x, dtype=np.float32))
    freqs_cis = np.asarray(freqs_cis, dtype=np.float32)
    wqkv = np.asarray(wqkv, dtype=np.float32)
    wo = np.asarray(wo, dtype=np.float32)

    perm = np.concatenate([np.arange(0, HD, 2), np.arange(1, HD, 2)])
    wq = wqkv[:D].reshape(16, HD, D)[:, perm, :]
    wk = wqkv[D:D + 512].reshape(4, HD, D)[:, perm, :]
    wv = wqkv[D + 512:].reshape(4, HD, D)

    cosT = freqs_cis[:, :, 0].T            # [64, S]
    sinT = freqs_cis[:, :, 1].T
    cos2 = _to_bf16(np.concatenate([cosT, cosT], axis=0))
    sinpm = _to_bf16(np.concatenate([-sinT, sinT], axis=0))

    xts = [_to_bf16(x[b].T) for b in range(NB)]
    in_maps = []
    for b in range(NB):
        for g in range(NG):
            wshard = np.concatenate(
                [wq[g * 4 + h] for h in range(4)] + [wk[g], wv[g]], axis=0)
            wqkvt = _to_bf16(wshard.T)
            wot = (_to_bf16(wo[:, g * 512:(g + 1) * 512].T) if proj_bf16
                   else np.ascontiguousarray(wo[:, g * 512:(g + 1) * 512].T))
            in_maps.append({"xt": xts[b], "wqkvt": wqkvt, "wot": wot,
                            "cos2": cos2, "sinpm": sinpm})
    return in_maps


def kernel(x, freqs_cis, wqkv, wo):
    if "main" not in _NC_CACHE:
        _NC_CACHE["main"] = build_nc()
    nc = _NC_CACHE["main"]
    in_maps = _prep_inputs(x, freqs_cis, wqkv, wo)
    res = run_bass_kernel_spmd(nc, in_maps, list(range(NB * NG)))
    out = np.zeros((NB, S, D), dtype=np.float32)
    for b in range(NB):
        for g in range(NG):
            out[b] += res.results[b * NG + g]["out"].astype(np.float32)
    return out
